# revision 47
# baseline (speedup 1.0000x reference)
"""Fused multi-head attention (RMSNorm-QK + RoPE + softmax + O-proj) on 8 TRN2 cores.

Sharding: tensor-parallel over heads (16 heads / 8 cores = 2 heads per core).
Each core computes Q/K/V projections for its 2 heads over all tokens, full
attention for those heads, and a partial O-projection (rows of wo for its
heads). Host sums the 8 partial outputs.

Everything on-device runs in the transposed [feature, token] layout so every
matmul has its contraction dim on partitions with no on-device transposes of
activations (only V needs a PE transpose). Softmax/RMSNorm partition-axis sums
use an all-ones stationary matrix (M=128), which also broadcasts the result to
all partitions, so no partition-broadcast ops are ever needed.
"""

import numpy as np
import ml_dtypes

import concourse.bass as bass
import concourse.tile as tile
from concourse import bacc, mybir
from concourse.bass_utils import run_bass_kernel_spmd

B, S, HID = 4, 2048, 2048
NH, HD = 16, 128
N_CORES = 8
HPC = NH // N_CORES          # heads per core = 2
KC = HID // 128              # 16 contraction chunks
TS = 512                     # free-dim tile (one PSUM bank of f32)
NTS = S // TS                # 4
TC = S // 128                # 16 token chunks of 128
EPS = 1e-6

BF16 = mybir.dt.bfloat16
F32 = mybir.dt.float32
AF = mybir.ActivationFunctionType
bf = ml_dtypes.bfloat16

_CACHE = {}


def _patch_act_tables():
    """Constrain exp/ln to the one ACT table set that holds both, so the
    table chooser stops flip-flopping between exp-only and ln-only sets
    (65 x 1283ns ACT_TABLE_LOADs otherwise). Set ids keep their act_info
    indices - only the advertised membership shrinks."""
    import concourse.bacc as bacc_mod
    import concourse.hw_specs as hw_specs_mod
    if getattr(bacc_mod, "_act_tables_patched", False):
        return
    orig = hw_specs_mod.get_activation_tables

    def patched(arch):
        tabs = orig(arch)
        keep = {"natural_log_exp_and_others"}
        strip = {AF.Exp, AF.Ln}
        return {
            name: (fns if name in keep else fns - strip)
            for name, fns in tabs.items()
        }

    bacc_mod.get_activation_tables = patched
    bacc_mod._act_tables_patched = True


def _build():
    _patch_act_tables()
    nc = bacc.Bacc("TRN2", target_bir_lowering=False, debug=False,
                   num_devices=N_CORES)

    xt_d = nc.dram_tensor("xt", [B, KC, 128, S], BF16, kind="ExternalInput").ap()
    wq_d = nc.dram_tensor("wq", [128, KC, HPC * HD], BF16, kind="ExternalInput").ap()
    wk_d = nc.dram_tensor("wk", [128, KC, HPC * HD], BF16, kind="ExternalInput").ap()
    wv_d = nc.dram_tensor("wv", [128, KC, HPC * HD], BF16, kind="ExternalInput").ap()
    wo_d = nc.dram_tensor("wo", [HPC, 128, HID], BF16, kind="ExternalInput").ap()
    cosq_d = nc.dram_tensor("cosq", [HD, S], BF16, kind="ExternalInput").ap()
    sinq_d = nc.dram_tensor("sinq", [HD, S], BF16, kind="ExternalInput").ap()
    cosk_d = nc.dram_tensor("cosk", [HD, S], BF16, kind="ExternalInput").ap()
    sink_d = nc.dram_tensor("sink", [HD, S], BF16, kind="ExternalInput").ap()
    out_d = nc.dram_tensor("out", [B, HID, S], BF16, kind="ExternalOutput").ap()

    ones_d = nc.inline_tensor(np.ones((128, 128), dtype=bf), name="ones_c").ap()

    with tile.TileContext(nc) as tc:
        _graph(nc, tc, xt_d, (wq_d, wk_d, wv_d), wo_d,
               (cosq_d, sinq_d, cosk_d, sink_d), ones_d, out_d)
    nc.compile()
    return nc


def _graph(nc, tc, xt_d, w_d, wo_d, tabs_d, ones_d, out_d):
    from contextlib import ExitStack
    ctx = ExitStack()
    with ctx:
        consts = ctx.enter_context(tc.tile_pool(name="consts", bufs=1))
        xt_pool = ctx.enter_context(tc.tile_pool(name="xt", bufs=29))
        raw_pool = ctx.enter_context(tc.tile_pool(name="raw", bufs=7))
        tmp_pool = ctx.enter_context(tc.tile_pool(name="tmp", bufs=2))
        hat_pool = ctx.enter_context(tc.tile_pool(name="hat", bufs=8))
        v_pool = ctx.enter_context(tc.tile_pool(name="v", bufs=4))
        es_pool = ctx.enter_context(tc.tile_pool(name="es", bufs=8))
        rec_pool = ctx.enter_context(tc.tile_pool(name="rec", bufs=2))
        o_pool = ctx.enter_context(tc.tile_pool(name="o", bufs=3))
        ost_pool = ctx.enter_context(tc.tile_pool(name="ost", bufs=2))
        pp_psum = ctx.enter_context(tc.tile_pool(name="pp", bufs=2, space="PSUM"))
        s_psum = ctx.enter_context(tc.tile_pool(name="sp", bufs=3, space="PSUM"))
        op_psum = ctx.enter_context(tc.tile_pool(name="op", bufs=2, space="PSUM"))
        sum_psum = ctx.enter_context(tc.tile_pool(name="sum", bufs=1, space="PSUM"))

        # ---- persistent constants ----
        wsb = []
        for i, wd in enumerate(w_d):
            t = consts.tile([128, KC, HPC * HD], BF16, tag=f"w{i}", name=f"w{i}")
            nc.sync.dma_start(out=t[:], in_=wd[:])
            wsb.append(t)
        wo_sb = consts.tile([128, HPC, HID], BF16, tag="wo", name="wo_sb")
        for h in range(HPC):
            nc.sync.dma_start(out=wo_sb[:, h, :], in_=wo_d[h])
        tabs = []
        for i, td in enumerate(tabs_d):
            t = consts.tile([HD, S], BF16, tag=f"tab{i}", name=f"tab{i}")
            nc.sync.dma_start(out=t[:], in_=td[:])
            tabs.append(t)
        cosq_sb, sinq_sb, cosk_sb, sink_sb = tabs
        ones_sb = consts.tile([128, 128], BF16, tag="ones", name="ones_sb")
        nc.sync.dma_start(out=ones_sb[:], in_=ones_d[:])
        eps_sb = consts.tile([128, 1], F32, tag="eps", name="eps_sb")
        nc.vector.memset(eps_sb[:], EPS)
        zero_sb = consts.tile([128, 1], F32, tag="zero", name="zero_sb")
        nc.vector.memset(zero_sb[:], 0.0)

        def norm_rope_slice(rawt, cos_sb, sin_sb, hatt, sl):
            """RMSNorm + RoPE for one t-slice, [d,t] layout.

            rstd is uniform over d so it commutes with rotate-half: the rope
            combination runs on the RAW projection (overlapping the ACT
            ln/exp rstd chain) and rstd multiplies once at the end."""
            sq = tmp_pool.tile([128, TS], BF16, tag="sq", name="sq")
            nc.vector.tensor_mul(sq[:], rawt[:, sl], rawt[:, sl])
            vps = pp_psum.tile([128, TS], F32, tag="pp", name="pp")
            nc.tensor.matmul(vps[:], ones_sb[:], sq[:], start=True, stop=True)
            lnt = tmp_pool.tile([128, TS], F32, tag="ln", name="lnt")
            nc.scalar.activation(lnt[:], vps[:], AF.Ln, scale=1.0 / HD, bias=eps_sb[:])
            rstd = tmp_pool.tile([128, TS], BF16, tag="rstd", name="rstd")
            nc.scalar.activation(rstd[:], lnt[:], AF.Exp, scale=-0.5, bias=zero_sb[:])
            rot = tmp_pool.tile([128, TS], BF16, tag="rot", name="rot")
            nc.vector.tensor_copy(rot[0:64, :], rawt[64:128, sl])
            nc.vector.tensor_copy(rot[64:128, :], rawt[0:64, sl])
            t1 = tmp_pool.tile([128, TS], BF16, tag="t1", name="t1")
            nc.vector.tensor_mul(t1[:], rawt[:, sl], cos_sb[:, sl])
            nc.vector.tensor_mul(rot[:], rot[:], sin_sb[:, sl])
            nc.vector.tensor_add(t1[:], t1[:], rot[:])
            nc.vector.tensor_mul(hatt[:, sl], t1[:], rstd[:])

        def emit_proj_ts(b, ts, raws, hats):
            """One t-slice of Q/K/V projections + norm/rope for that slice."""
            with nc.named_scope(f"proj_b{b}t{ts}"):
                sl = slice(ts * TS, (ts + 1) * TS)
                xts = []
                for kc in range(KC):
                    xtile = xt_pool.tile([128, TS], BF16, tag="xt", name="xtile")
                    nc.sync.dma_start(out=xtile[:], in_=xt_d[b, kc, :, sl])
                    xts.append(xtile)
                for h in range(HPC):
                    for pi in range(3):
                        ps = pp_psum.tile([128, TS], F32, tag="pp", name="pp")
                        for kc in range(KC):
                            nc.tensor.matmul(
                                ps[:], wsb[pi][:, kc, h * HD:(h + 1) * HD],
                                xts[kc][:], start=(kc == 0), stop=(kc == KC - 1))
                        nc.scalar.activation(raws[(pi, h)][:, sl], ps[:], AF.Copy)
                for h in range(HPC):
                    norm_rope_slice(raws[(0, h)], cosq_sb, sinq_sb,
                                    hats[(0, h)], sl)
                    norm_rope_slice(raws[(1, h)], cosk_sb, sink_sb,
                                    hats[(1, h)], sl)

        def emit_vtrans(b, h, rawv):
            # [d,t] -> [t,d] via DMA xbar transpose. The ~1.2us ucode runs on
            # the Scalar HWDGE queue so it doesn't clog Sync's xt-load queue,
            # and all of a batch's transposes are emitted back-to-back at the
            # end of its own wave (data ready, overlaps O-proj PE work).
            with nc.named_scope(f"vtrans_b{b}h{h}"):
                vt = v_pool.tile([128, TC, HD], BF16, tag="v", name="vt")
                for t in range(TC):
                    nc.sync.dma_start(out=vt[:, t, :],
                                      in_=rawv[:, t * 128:(t + 1) * 128],
                                      transpose=True)
                return vt

        def emit_attn_sc(b, h, sc, qhat, khat, vt, onorm):
            with nc.named_scope(f"attn_b{b}h{h}s{sc}"):
                ssl = slice(sc * TS, (sc + 1) * TS)
                osum = op_psum.tile([128, TS], F32, tag="ops", name="osum")
                ssum = sum_psum.tile([128, TS], F32, tag="ssum", name="ssum")
                ess = [None] * 4
                for t in range(TC):
                    sps = s_psum.tile([128, TS], F32, tag="sps", name="sps")
                    nc.tensor.matmul(
                        sps[:], khat[:, t * 128:(t + 1) * 128],
                        qhat[:, ssl], start=True, stop=True)
                    es = es_pool.tile([128, TS], BF16, tag="es", name="es")
                    nc.scalar.activation(es[:], sps[:], AF.Exp, bias=zero_sb[:])
                    nc.tensor.matmul(osum[:], vt[:, t, :], es[:],
                                     start=(t == 0), stop=(t == TC - 1))
                    # softmax denominator: pre-add quads of es on DVE so the
                    # ones-matmul partition-sum runs 4x per s-chunk, not 16x
                    j = t % 4
                    r = t // 4
                    ess[j] = es
                    if j == 1 or j == 3:
                        ep = es_pool.tile([128, TS], BF16, tag="ep", name="ep", bufs=3)
                        nc.vector.tensor_add(ep[:], ess[j - 1][:], ess[j][:])
                        ess[j] = ep
                    if j == 3:
                        eq = es_pool.tile([128, TS], BF16, tag="eq", name="eq", bufs=3)
                        nc.vector.tensor_add(eq[:], ess[1][:], ess[3][:])
                        nc.tensor.matmul(ssum[:], ones_sb[:], eq[:],
                                         start=(r == 0), stop=(r == 3))
                rec = rec_pool.tile([128, TS], F32, tag="rec", name="rec")
                nc.vector.reciprocal(rec[:], ssum[:])
                nc.vector.tensor_mul(onorm[:, ssl], osum[:], rec[:])

        def emit_oproj(b, onorms):
            for mc in range(KC):
                with nc.named_scope(f"oproj_b{b}m{mc}"):
                    ostage = ost_pool.tile([128, S], BF16, tag="ost",
                                           name="ostage")
                    for sc in range(NTS):
                        ssl = slice(sc * TS, (sc + 1) * TS)
                        pso = pp_psum.tile([128, TS], F32, tag="pp", name="pso")
                        for h in range(HPC):
                            nc.tensor.matmul(
                                pso[:], wo_sb[:, h, mc * 128:(mc + 1) * 128],
                                onorms[h][:, ssl],
                                start=(h == 0), stop=(h == HPC - 1))
                        nc.vector.tensor_copy(ostage[:, ssl], pso[:])
                    nc.sync.dma_start(out=out_d[b, mc * 128:(mc + 1) * 128, :],
                                      in_=ostage[:])

        def emit_oproj_sc(b, onorms, sc):
            ssl = slice(sc * TS, (sc + 1) * TS)
            with nc.named_scope(f"oprojsc_b{b}s{sc}"):
                for mc in range(KC):
                    pso = pp_psum.tile([128, TS], F32, tag="pp", name="pso")
                    for h in range(HPC):
                        nc.tensor.matmul(
                            pso[:], wo_sb[:, h, mc * 128:(mc + 1) * 128],
                            onorms[h][:, ssl],
                            start=(h == 0), stop=(h == HPC - 1))
                    ost2 = ost_pool.tile([128, TS], BF16, tag="ost2",
                                         name="ost2", bufs=4)
                    nc.vector.tensor_copy(ost2[:], pso[:])
                    nc.sync.dma_start(
                        out=out_d[b, mc * 128:(mc + 1) * 128, ssl], in_=ost2[:])

        # Software pipeline across batches: batch b's projections (PE-dense,
        # ACT-light) interleave with batch b-1's attention (ACT-dense,
        # PE-light) so neither engine drains the other.
        prev = None
        for b in range(B + 1):
            cur = None
            if b < B:
                raws = {}
                hats = {}
                for h in range(HPC):
                    for pi in range(3):
                        raws[(pi, h)] = raw_pool.tile(
                            [128, S], BF16, tag="raw", name="rawt")
                    for qk in range(2):
                        hats[(qk, h)] = hat_pool.tile(
                            [128, S], BF16, tag="hat", name="hatt")
                cur = (raws, hats)

            attn_state = None
            if prev is not None:
                pb, phats, pvts = prev
                onorms = {h: o_pool.tile([128, S], BF16, tag="on", name="onorm")
                          for h in range(HPC)}
                attn_state = (pb, phats, onorms, pvts)

            if b == B and attn_state is not None:
                # Epilogue wave: no projections to interleave, so weave the
                # O-projection in per s-chunk to keep PE busy between the
                # ACT-paced attention chunks.
                pb, phats, onorms, vts = attn_state
                for sc in range(NTS):
                    for h in range(HPC):
                        emit_attn_sc(pb, h, sc, phats[(0, h)], phats[(1, h)],
                                     vts[h], onorms[h])
                    emit_oproj_sc(pb, onorms, sc)
            else:
                for step in range(NTS):
                    if attn_state is not None:
                        pb, phats, onorms, vts = attn_state
                        h = step // 2
                        for sc in (0, 1) if step % 2 == 0 else (2, 3):
                            emit_attn_sc(pb, h, sc, phats[(0, h)],
                                         phats[(1, h)], vts[h], onorms[h])
                    if b < B:
                        emit_proj_ts(b, step, cur[0], cur[1])
                # V transposes for batch b, emitted back-to-back at wave end
                # so they're done before b's attention starts next wave
                vts_new = None
                if b < B:
                    vts_new = {h: emit_vtrans(b, h, cur[0][(2, h)])
                               for h in range(HPC)}
                if attn_state is not None:
                    emit_oproj(attn_state[0], attn_state[2])

            if b < B:
                prev = (b, cur[1], vts_new)
            else:
                prev = None


def _prep_inputs(hidden_states, cos, sin, wq, wk, wv, wo, q_norm_w, k_norm_w):
    hs = np.asarray(hidden_states, np.float32)
    cos = np.asarray(cos, np.float32)
    sin = np.asarray(sin, np.float32)
    wq = np.asarray(wq, np.float32)
    wk = np.asarray(wk, np.float32)
    wv = np.asarray(wv, np.float32)
    wo = np.asarray(wo, np.float32)
    q_norm_w = np.asarray(q_norm_w, np.float32)
    k_norm_w = np.asarray(k_norm_w, np.float32)

    xt = np.ascontiguousarray(
        hs.transpose(0, 2, 1).reshape(B, KC, 128, S).astype(bf))

    sign = np.concatenate([-np.ones(HD // 2, np.float32),
                           np.ones(HD // 2, np.float32)])

    def make_tabs(w, scale):
        wsh = np.concatenate([w[HD // 2:], w[:HD // 2]])
        cosT = np.ascontiguousarray((cos.T * (w * scale)[:, None]).astype(bf))
        sinT = np.ascontiguousarray(
            (sin.T * (wsh * sign * scale)[:, None]).astype(bf))
        return cosT, sinT

    cosq, sinq = make_tabs(q_norm_w, HD ** -0.5)
    cosk, sink = make_tabs(k_norm_w, 1.0)

    def pack_w(w, c):
        wc = w[:, c * HPC * HD:(c + 1) * HPC * HD]
        return np.ascontiguousarray(
            wc.reshape(KC, 128, HPC * HD).transpose(1, 0, 2).astype(bf))

    in_maps = []
    for c in range(N_CORES):
        wo_c = np.ascontiguousarray(
            wo[c * HPC * HD:(c + 1) * HPC * HD, :].reshape(HPC, 128, HID).astype(bf))
        in_maps.append({
            "xt": xt,
            "wq": pack_w(wq, c), "wk": pack_w(wk, c), "wv": pack_w(wv, c),
            "wo": wo_c,
            "cosq": cosq, "sinq": sinq, "cosk": cosk, "sink": sink,
        })
    return in_maps


LAST_RESULTS = None


def kernel(hidden_states, cos, sin, attention_mask, wq, wk, wv, wo,
           q_norm_w, k_norm_w, _trace=False):
    global LAST_RESULTS
    if "nc" not in _CACHE:
        _CACHE["nc"] = _build()
    nc = _CACHE["nc"]
    in_maps = _prep_inputs(hidden_states, cos, sin, wq, wk, wv, wo,
                           q_norm_w, k_norm_w)
    res = run_bass_kernel_spmd(nc, in_maps, core_ids=list(range(N_CORES)),
                               trace=_trace)
    LAST_RESULTS = res
    acc = np.zeros((B, HID, S), np.float32)
    for r in res.results:
        acc += r["out"].astype(np.float32)
    return np.ascontiguousarray(acc.transpose(0, 2, 1))
